# revision 32
# baseline (speedup 1.0000x reference)
"""EpisodicMemory kernel for Trainium2, data-parallel over batch on 8 NeuronCores.

Per-core computation (one batch element b, S=4096, D=1024, M=64, H=4, DH=256):

Host-side algebraic fusion (exact linear algebra, fp64 numpy):
  k        = mk @ wk.T + bk                              (M, D)
  FUSED_K  = stack_h[(k_h @ wq_h) / sqrt(DH)]            (H*M, D)
  scores   = x @ FUSED_K.T + sbias        (replaces q-proj + qk matmul)
  BIG_W    = [mk | wg | FUSED_K]                         (M+1+H*M, D)
  fused2   = comb_w[:, D:] @ out_w                       (D, D)
  cw1      = comb_w[:, :D]                               (D, D)
  combb    = comb_b + comb_w[:, D:] @ out_b              (D,)

Device algebra: fold fused2 into the value path per head:
  VF[(h,m), :] = v[m, hDH:(h+1)DH] @ fused2[:, hDH:(h+1)DH].T   (H*M, D)
  y = x @ cw1.T + P @ VF + combb     where P = concat_h softmax_h(scores)

Device phases (per core):
  1. per s-chunk (128 rows): pbig = x8 @ BIG_W8.T in fp8 DoubleRow (the
     write/read attention logits tolerate fp8; read path contributes ~6%
     of y). Exp-only ACT (tiny logits -> no max subtraction; sigmoid via
     exp(-z)), segmented per-head sums on DVE, P transposed on PE with one
     batched PSUM->SBUF copy. W accumulated pairwise in fp8 DoubleRow as
     gated.T @ [x8 | 1]. Three-deep software pipeline.
  2. slot_gate = min(colsum, 1); mv = slot_gate * W; v = mv @ wv.T + bv;
     VF per head via batched PE transposes + small matmuls.
  3. transposed output (bf16): for each d-chunk, yT[d, :] accumulates
     cw1T-chunks.T @ xT-stream + VF-chunks.T @ pT-stream in PSUM, + combb.
     This x @ cw1 part stays bf16 (precision-critical). Host transposes back.
Weights are host-pre-shuffled to partition-major layouts for contiguous DMA;
late-phase weights and xT(bf16) stream in behind phase 1.
"""

import numpy as np
import ml_dtypes

import concourse.bass as bass
import concourse.mybir as mybir
import concourse.tile as tile
from concourse import bacc
from concourse.bass_utils import run_bass_kernel_spmd
from concourse.masks import make_identity

F32 = mybir.dt.float32
BF16 = mybir.dt.bfloat16
FP8 = mybir.dt.float8e4
AX = mybir.AxisListType.X
AF = mybir.ActivationFunctionType
ALU = mybir.AluOpType
DR = mybir.MatmulPerfMode.DoubleRow

B, D, M, H = 8, 1024, 64, 4
DH = D // H
GW = M + 1 + H * M   # 321 columns of BIG_W output
GWP = 336            # padded so the fp8 DoubleRow Ko stride is 16-aligned
N_CORES = 8


def build_program(S=4096, add_sbias=False):
    NCH = S // 128   # s-chunks
    NT = S // 512    # s-tiles
    DC = D // 128    # d-chunks

    nc = bacc.Bacc(None, target_bir_lowering=False, debug=False)

    x8_d = nc.dram_tensor("x8", [S, D], FP8, kind="ExternalInput")
    xT8_d = nc.dram_tensor("xT8", [128, NT * DC * 512], FP8,
                           kind="ExternalInput")
    xT_d = nc.dram_tensor("xT", [D, S], BF16, kind="ExternalInput")
    bigw8_d = nc.dram_tensor("bigw8", [128, DC * GWP], FP8,
                             kind="ExternalInput")
    wvT_d = nc.dram_tensor("wvT", [128, DC * D], BF16, kind="ExternalInput")
    f2T_d = nc.dram_tensor("f2T", [128, DC * D], BF16, kind="ExternalInput")
    cw1T_d = nc.dram_tensor("cw1T", [128, DC * D], BF16, kind="ExternalInput")
    bv_d = nc.dram_tensor("bv", [D], F32, kind="ExternalInput")
    combb_d = nc.dram_tensor("combb", [D], F32, kind="ExternalInput")
    wgbn_d = nc.dram_tensor("wgbn", [1], F32, kind="ExternalInput")
    sbias_d = nc.dram_tensor("sbias", [H * M], F32, kind="ExternalInput")
    yT_d = nc.dram_tensor("yT", [D, S], BF16, kind="ExternalOutput")

    x8_ap = x8_d.ap()
    yT_ap = yT_d.ap()
    xT8_r = xT8_d.ap().rearrange("p (t dc s) -> p t dc s", t=NT, dc=DC)
    xT_r = xT_d.ap().rearrange("(dc p) s -> p dc s", p=128)
    bigw8_r = bigw8_d.ap().rearrange("p (dc g) -> p dc g", dc=DC)
    wvT_r = wvT_d.ap().rearrange("p (dc g) -> p dc g", dc=DC)
    f2T_r = f2T_d.ap().rearrange("p (dc g) -> p dc g", dc=DC)
    cw1T_r = cw1T_d.ap().rearrange("p (dc g) -> p dc g", dc=DC)
    combb_r = combb_d.ap().rearrange("(dc p) -> p dc", p=128)

    def bcast(ap, n):
        return bass.AP(tensor=ap.tensor, offset=ap.offset, ap=[[0, n]] + list(ap.ap))

    with tile.TileContext(nc) as tc:
        with tc.tile_pool(name="singles", bufs=1) as singles:
            # phase-1-critical loads first, on the sync queue
            bigw8_sb = singles.tile([128, DC, GWP], FP8)
            nc.sync.dma_start(bigw8_sb, bigw8_r)
            xT8_sb = singles.tile([128, NT, DC, 512], FP8)
            for t0 in range(3):
                nc.sync.dma_start(xT8_sb[:, t0], xT8_r[:, t0])
            # small constants on gpsimd queue
            combb_sb = singles.tile([128, DC], F32)
            nc.gpsimd.dma_start(combb_sb, combb_r)
            bvb_sb = singles.tile([64, D], F32)
            nc.gpsimd.dma_start(bvb_sb, bcast(bv_d.ap(), 64))
            wgbn_sb = singles.tile([128, 1], F32)
            nc.gpsimd.dma_start(wgbn_sb, bcast(wgbn_d.ap(), 128))
            sbias_sb = singles.tile([128, H * M], F32)
            nc.gpsimd.dma_start(sbias_sb, bcast(sbias_d.ap(), 128))
            ident = singles.tile([128, 128], BF16)
            make_identity(nc, ident)
            onesp = singles.tile([128, 2, 16], FP8)
            nc.vector.memset(onesp, 1.0)
            pT_all = singles.tile([128, 2, S], FP8)
            # deferred big loads (issued inside phase 1, gpsimd queue)
            xT_sb = singles.tile([128, DC, S], BF16)
            cw1T_sb = singles.tile([128, DC, D], BF16)
            wvT_sb = singles.tile([128, DC, D], BF16)
            f2T_sb = singles.tile([128, DC, D], BF16)

            # ---------------- phase 1: write-attention ----------------
            with (
                tc.tile_pool(name="ps1", bufs=1, space="PSUM") as ps1,
                tc.tile_pool(name="xin", bufs=3) as xin,
                tc.tile_pool(name="wk1", bufs=4) as wk1,
            ):
                ps_w = ps1.tile([64, 1536], F32, tag="w")

                def issue_pbig(c):
                    t, cc = c // 4, c % 4
                    pbig = ps1.tile([128, GW], F32, tag="big", bufs=3)
                    for j in range(DC // 2):
                        nc.tensor.matmul(
                            pbig,
                            lhsT=xT8_sb[:, t, 2 * j:2 * j + 2,
                                        cc * 128:(cc + 1) * 128],
                            rhs=bigw8_sb[:, 2 * j:2 * j + 2, 0:GW],
                            perf_mode=DR,
                            start=(j == 0), stop=(j == DC // 2 - 1),
                        )
                    return pbig

                def exps(c, pbig, pt):
                    es2, eg2, eh2 = pt
                    j = c % 2
                    if add_sbias:
                        nc.vector.tensor_add(
                            pbig[:, M + 1:GW], pbig[:, M + 1:GW], sbias_sb
                        )
                    nc.scalar.activation(es2[:, j, :], pbig[:, 0:M], AF.Exp)
                    nc.scalar.activation(eg2[:, j:j + 1], pbig[:, M:M + 1],
                                         AF.Exp, scale=-1.0, bias=wgbn_sb)
                    nc.scalar.activation(eh2[:, j], pbig[:, M + 1:GW], AF.Exp)

                def dve_pair(cp, pt, gcp):
                    es2, eg2, eh2 = pt
                    esum2 = wk1.tile([128, 2], F32, tag="esum")
                    nc.vector.reduce_sum(esum2, es2, axis=AX)
                    hs2 = wk1.tile([128, 2, H], F32, tag="hs")
                    nc.vector.reduce_sum(hs2, eh2, axis=AX)
                    den2 = wk1.tile([128, 2], F32, tag="den")
                    nc.vector.scalar_tensor_tensor(
                        out=den2, in0=eg2, scalar=1.0, in1=esum2,
                        op0=ALU.add, op1=ALU.mult)
                    r2 = wk1.tile([128, 2], F32, tag="r2")
                    nc.vector.reciprocal(r2, den2)
                    rh2 = wk1.tile([128, 2, H], F32, tag="rh")
                    nc.vector.reciprocal(rh2, hs2)
                    nc.gpsimd.tensor_mul(gcp, es2,
                                         r2.broadcast_to((128, 2, M)))
                    pn2 = wk1.tile([128, 2, H, M], BF16, tag="pn")
                    nc.vector.tensor_mul(pn2, eh2,
                                         rh2.broadcast_to((128, 2, H, M)))
                    return pn2

                def trans_pair(cp, pn2):
                    pnf = pn2.rearrange("p j h m -> p (j h m)")
                    ptr = ps1.tile([128, 512], BF16, tag="tr", bufs=2)
                    for jq in range(4):
                        nc.tensor.transpose(
                            ptr[:, jq * 128:(jq + 1) * 128],
                            pnf[:, jq * 128:(jq + 1) * 128], ident)
                    nc.vector.tensor_copy(
                        pT_all[:, :, cp * 256:(cp + 1) * 256].rearrange(
                            "p q (j u) -> p q j u", j=2),
                        ptr.rearrange("p (j q u) -> p q j u", j=2, q=2))

                def pe_tail(cp, gcp, xp):
                    st, sp = (cp == 0), (cp == NCH // 2 - 1)
                    nc.tensor.matmul(ps_w[:, 0:512], lhsT=gcp,
                                     rhs=xp[:, :, 0:512], perf_mode=DR,
                                     start=st, stop=sp)
                    nc.tensor.matmul(ps_w[:, 512:1024], lhsT=gcp,
                                     rhs=xp[:, :, 512:1024], perf_mode=DR,
                                     start=st, stop=sp)
                    nc.tensor.matmul(ps_w[:, 1024:1025], lhsT=gcp,
                                     rhs=onesp[:, :, 0:1], perf_mode=DR,
                                     start=st, stop=sp)

                # deferred-load schedule: (chunk -> dma issue) on the ACT
                # queue's DMA ring so the sync ring (xp/xT8) is undisturbed;
                # xT tiles 4-7 and cw1T stream during boundary/early phase 2
                def deferred_loads(c):
                    if c in (3, 7, 11, 15):
                        t = (c - 3) // 4
                        nc.scalar.dma_start(
                            xT_sb[:, :, t * 512:(t + 1) * 512],
                            xT_r[:, :, t * 512:(t + 1) * 512])
                    elif c in (19, 21):
                        h = (c - 19) // 2
                        nc.scalar.dma_start(
                            wvT_sb[:, 4 * h:4 * h + 4, :],
                            wvT_r[:, 4 * h:4 * h + 4, :])
                    elif c in (23, 25):
                        h = (c - 23) // 2
                        nc.scalar.dma_start(
                            f2T_sb[:, 4 * h:4 * h + 4, :],
                            f2T_r[:, 4 * h:4 * h + 4, :])
                    elif c in (27, 29):
                        h = (c - 27) // 2
                        nc.scalar.dma_start(
                            cw1T_sb[:, 4 * h:4 * h + 4, :],
                            cw1T_r[:, 4 * h:4 * h + 4, :])

                pairs = {}
                ptiles = {}
                chunks = {}
                pns = {}
                for c in range(NCH):
                    if c % 4 == 0 and c // 4 + 3 < NT:
                        t = c // 4 + 3
                        nc.sync.dma_start(xT8_sb[:, t], xT8_r[:, t])
                    if c % 2 == 0:
                        cp = c // 2
                        xp = xin.tile([128, 2, D], FP8, tag="xp")
                        nc.sync.dma_start(
                            xp, x8_ap[cp * 256:(cp + 1) * 256, :].rearrange(
                                "(j p) d -> p j d", p=128))
                        gcp = wk1.tile([128, 2, M], FP8, tag="gcp", bufs=3)
                        pairs[cp] = (gcp, xp)
                        es2 = wk1.tile([128, 2, M], BF16, tag="es", bufs=3)
                        eg2 = wk1.tile([128, 2], F32, tag="eg", bufs=3)
                        eh2 = wk1.tile([128, 2, H, M], BF16, tag="eh", bufs=3)
                        ptiles[cp] = (es2, eg2, eh2)
                    pbig = issue_pbig(c)
                    chunks[c] = pbig
                    if c >= 1:
                        exps(c - 1, chunks[c - 1], ptiles[(c - 1) // 2])
                        del chunks[c - 1]
                    if c >= 2 and c % 2 == 0:
                        cp = (c - 2) // 2
                        pns[cp] = dve_pair(cp, ptiles[cp], pairs[cp][0])
                        del ptiles[cp]
                    if c >= 3 and c % 2 == 1:
                        cp = (c - 3) // 2
                        trans_pair(cp, pns[cp])
                        del pns[cp]
                    deferred_loads(c)
                    if c >= 4 and c % 2 == 0:
                        cp = (c - 4) // 2
                        pe_tail(cp, *pairs[cp])
                        del pairs[cp]
                LP = NCH // 2 - 1  # last pair
                exps(NCH - 1, chunks[NCH - 1], ptiles[LP])
                pns[LP] = dve_pair(LP, ptiles[LP], pairs[LP][0])
                trans_pair(LP, pns[LP])
                pe_tail(LP - 1, *pairs[LP - 1])
                pe_tail(LP, *pairs[LP])

                # --- slot gate ---
                ssum = singles.tile([64, 1], F32)
                nc.vector.tensor_copy(ssum, ps_w[:, 1024:1025])
                sg = singles.tile([64, 1], F32)
                nc.vector.tensor_scalar_min(sg, ssum, 1.0)
                mv_bf = singles.tile([64, D], BF16)
                nc.vector.tensor_scalar_mul(mv_bf, ps_w[:, 0:D], sg)

            # ---------------- phase boundary: v and VF ----------------
            mvT_sb = singles.tile([128, DC, 64], BF16)
            vT_sb = singles.tile([128, DC, 64], BF16)
            v_sb = singles.tile([64, D], BF16)
            vf8 = singles.tile([128, 2, D], FP8)
            with tc.tile_pool(name="psB", bufs=1, space="PSUM") as psB:
                trB0 = psB.tile([128, DC, 64], BF16, tag="trb", bufs=2)
                for dc in range(DC):
                    nc.tensor.transpose(
                        trB0[:, dc, :], mv_bf[:, dc * 128:(dc + 1) * 128],
                        ident[0:64, 0:64])
                nc.vector.tensor_copy(mvT_sb, trB0)
                pv = psB.tile([64, D], F32, tag="v")
                for g2 in range(2):
                    for dc in range(DC):
                        nc.tensor.matmul(
                            pv[:, g2 * 512:(g2 + 1) * 512],
                            lhsT=mvT_sb[:, dc, :],
                            rhs=wvT_sb[:, dc, g2 * 512:(g2 + 1) * 512],
                            start=(dc == 0), stop=(dc == DC - 1),
                        )
                nc.vector.tensor_add(v_sb, pv, bvb_sb)
                trB1 = psB.tile([128, DC, 64], BF16, tag="trb", bufs=2)
                for dc in range(DC):
                    nc.tensor.transpose(
                        trB1[:, dc, :], v_sb[:, dc * 128:(dc + 1) * 128],
                        ident[0:64, 0:64])
                nc.vector.tensor_copy(vT_sb, trB1)
                for q in range(2):
                    pvf = psB.tile([128, D], F32, tag="vf", bufs=2)
                    for hh in range(2):
                        h = 2 * q + hh
                        for cc in range(2):
                            for g2 in range(2):
                                nc.tensor.matmul(
                                    pvf[hh * 64:(hh + 1) * 64,
                                        g2 * 512:(g2 + 1) * 512],
                                    lhsT=vT_sb[:, h * 2 + cc, :],
                                    rhs=f2T_sb[:, h * 2 + cc,
                                               g2 * 512:(g2 + 1) * 512],
                                    start=(cc == 0), stop=(cc == 1),
                                )
                    nc.vector.tensor_copy(vf8[:, q, :], pvf)

            # ---------------- phase 2: transposed output ----------------
            with (
                tc.tile_pool(name="ps2", bufs=2, space="PSUM") as ps2,
                tc.tile_pool(name="wk2", bufs=4) as wk2,
            ):
                for sh in range(2):
                    for dc in range(DC):
                        if sh == 0 and dc in (0, 2, 4, 6):
                            t = 4 + dc // 2
                            nc.gpsimd.dma_start(
                                xT_sb[:, :, t * 512:(t + 1) * 512],
                                xT_r[:, :, t * 512:(t + 1) * 512])
                        po = [ps2.tile([128, 1024], F32, tag=f"o{i}",
                                       name=f"po{i}") for i in range(2)]
                        for ci in range(DC + 1):
                            for st in range(4):
                                s0 = sh * 2048 + st * 512
                                out = po[st // 2][:, (st % 2) * 512:
                                                  (st % 2) * 512 + 512]
                                if ci < DC:
                                    nc.tensor.matmul(
                                        out,
                                        lhsT=cw1T_sb[:, ci,
                                                     dc * 128:(dc + 1) * 128],
                                        rhs=xT_sb[:, ci, s0:s0 + 512],
                                        start=(ci == 0), stop=False,
                                    )
                                else:
                                    nc.tensor.matmul(
                                        out,
                                        lhsT=vf8[:, :, dc * 128:(dc + 1) * 128],
                                        rhs=pT_all[:, :, s0:s0 + 512],
                                        perf_mode=DR,
                                        start=False, stop=True,
                                    )
                        ytile = wk2.tile([128, 4, 512], BF16, tag="yt")
                        for i in range(2):
                            nc.vector.tensor_scalar_add(
                                ytile[:, 2 * i:2 * i + 2, :], po[i],
                                combb_sb[:, dc:dc + 1])
                        nc.sync.dma_start(
                            yT_ap[dc * 128:(dc + 1) * 128,
                                  sh * 2048:(sh + 1) * 2048],
                            ytile,
                        )

    nc.compile()
    return nc


def prep_inputs(inputs, S=4096):
    """Host-side fusion + per-core shard maps."""
    f64 = np.float64
    bf = ml_dtypes.bfloat16
    f8 = ml_dtypes.float8_e4m3
    NT, DC = S // 512, D // 128
    x = np.asarray(inputs["x"], np.float32)
    mk = np.asarray(inputs["memory_keys"], np.float32)
    wg_w = np.asarray(inputs["wg_w"], np.float32)
    wg_b = np.asarray(inputs["wg_b"], np.float32)
    ipw = np.asarray(inputs["in_proj_w"], np.float32)
    ipb = np.asarray(inputs["in_proj_b"], np.float32)
    out_w = np.asarray(inputs["out_w"], np.float32)
    out_b = np.asarray(inputs["out_b"], np.float32)
    comb_w = np.asarray(inputs["comb_w"], np.float32)
    comb_b = np.asarray(inputs["comb_b"], np.float32)

    wq, wk, wv = ipw[:D], ipw[D:2 * D], ipw[2 * D:]
    bq, bk, bv = ipb[:D], ipb[D:2 * D], ipb[2 * D:]

    k_full = mk.astype(f64) @ wk.astype(f64).T + bk.astype(f64)      # (M, D)
    kh = k_full.reshape(M, H, DH)
    wqh = wq.astype(f64).reshape(H, DH, D)
    scl = 1.0 / np.sqrt(DH)
    FK = (np.einsum("mhd,hde->hme", kh, wqh) * scl).reshape(H * M, D)
    sbias = (np.einsum("hd,mhd->hm", bq.astype(f64).reshape(H, DH), kh)
             * scl).reshape(H * M)
    BIG_W = np.concatenate([mk.astype(f64), wg_w.astype(f64), FK], axis=0)

    fused2 = comb_w[:, D:].astype(f64) @ out_w.astype(f64)           # (D, D)
    combb = comb_b.astype(f64) + comb_w[:, D:].astype(f64) @ out_b.astype(f64)

    def preshuffle(wT, pad_to=None):
        # (D, G) -> (128, nc_*G): [p, dc*G+g] = wT[dc*128+p, g]
        nc_ = wT.shape[0] // 128
        G = wT.shape[1]
        if pad_to is not None and pad_to > G:
            wT = np.concatenate(
                [wT, np.zeros((wT.shape[0], pad_to - G), wT.dtype)], axis=1)
            G = pad_to
        return np.ascontiguousarray(
            wT.reshape(nc_, 128, G).transpose(1, 0, 2).reshape(128, nc_ * G))

    shared = {
        "bigw8": preshuffle(BIG_W.T.astype(np.float32),
                            pad_to=GWP).astype(f8),
        "wvT": preshuffle(np.ascontiguousarray(wv.T)).astype(bf),
        "f2T": preshuffle(np.ascontiguousarray(fused2.T)).astype(bf),
        "cw1T": preshuffle(np.ascontiguousarray(comb_w[:, :D].T)).astype(bf),
        "bv": bv.astype(np.float32),
        "combb": combb.astype(np.float32),
        "wgbn": (-wg_b).astype(np.float32),
        "sbias": sbias.astype(np.float32),
    }
    add_sbias = bool(np.any(shared["sbias"] != 0))

    in_maps = []
    for b in range(B):
        xb = x[b, :S]
        xbT = np.ascontiguousarray(xb.T)                  # (D, S)
        # xT8 layout [p, t, dc, s']: value = xT[dc*128+p, t*512+s']
        xT8 = (xbT.reshape(DC, 128, NT, 512)
               .transpose(1, 2, 0, 3).reshape(128, NT * DC * 512))
        m = dict(shared)
        m["x8"] = xb.astype(f8)
        m["xT8"] = np.ascontiguousarray(xT8).astype(f8)
        m["xT"] = xbT.astype(bf)
        in_maps.append(m)
    return in_maps, add_sbias


def kernel(_trace=False, _S=4096, **inputs):
    in_maps, add_sbias = prep_inputs(inputs, S=_S)
    nc = build_program(S=_S, add_sbias=add_sbias)
    kw = {}
    if _trace:
        kw = dict(trace=True, trace_cores=list(range(N_CORES)))
    res = run_bass_kernel_spmd(nc, in_maps, list(range(N_CORES)), **kw)
    y = np.stack(
        [np.asarray(res.results[i]["yT"]).astype(np.float32).T
         for i in range(N_CORES)],
        axis=0,
    )
    if _trace:
        return y, res
    return y


# revision 33
# speedup vs baseline: 1.0501x; 1.0501x over previous
"""EpisodicMemory kernel for Trainium2, data-parallel over batch on 8 NeuronCores.

Per-core computation (one batch element b, S=4096, D=1024, M=64, H=4, DH=256):

Host-side algebraic fusion (exact linear algebra, fp64 numpy):
  k        = mk @ wk.T + bk                              (M, D)
  FUSED_K  = stack_h[(k_h @ wq_h) / sqrt(DH)]            (H*M, D)
  scores   = x @ FUSED_K.T + sbias        (replaces q-proj + qk matmul)
  BIG_W    = [mk | wg | FUSED_K]                         (M+1+H*M, D)
  fused2   = comb_w[:, D:] @ out_w                       (D, D)
  cw1      = comb_w[:, :D]                               (D, D)
  combb    = comb_b + comb_w[:, D:] @ out_b              (D,)

Device algebra: fold fused2 into the value path per head:
  VF[(h,m), :] = v[m, hDH:(h+1)DH] @ fused2[:, hDH:(h+1)DH].T   (H*M, D)
  y = x @ cw1.T + P @ VF + combb     where P = concat_h softmax_h(scores)

Device phases (per core):
  1. per s-chunk (128 rows): pbig = x8 @ BIG_W8.T in fp8 DoubleRow (the
     write/read attention logits tolerate fp8; read path contributes ~6%
     of y). Exp-only ACT (tiny logits -> no max subtraction; sigmoid via
     exp(-z)), segmented per-head sums on DVE, P transposed on PE with one
     batched PSUM->SBUF copy. W accumulated pairwise in fp8 DoubleRow as
     gated.T @ [x8 | 1]. Three-deep software pipeline.
  2. slot_gate = min(colsum, 1); mv = slot_gate * W; v = mv @ wv.T + bv;
     VF per head via batched PE transposes + small matmuls.
  3. transposed output (bf16): for each d-chunk, yT[d, :] accumulates
     cw1T-chunks.T @ xT-stream + VF-chunks.T @ pT-stream in PSUM, + combb.
     This x @ cw1 part stays bf16 (precision-critical). Host transposes back.
Weights are host-pre-shuffled to partition-major layouts for contiguous DMA;
late-phase weights and xT(bf16) stream in behind phase 1.
"""

import numpy as np
import ml_dtypes

import concourse.bass as bass
import concourse.mybir as mybir
import concourse.tile as tile
from concourse import bacc
from concourse.bass_utils import run_bass_kernel_spmd
from concourse.masks import make_identity

F32 = mybir.dt.float32
BF16 = mybir.dt.bfloat16
FP8 = mybir.dt.float8e4
AX = mybir.AxisListType.X
AF = mybir.ActivationFunctionType
ALU = mybir.AluOpType
DR = mybir.MatmulPerfMode.DoubleRow

B, D, M, H = 8, 1024, 64, 4
DH = D // H
GW = M + 1 + H * M   # 321 columns of BIG_W output
GWP = 336            # padded so the fp8 DoubleRow Ko stride is 16-aligned
N_CORES = 8


def build_program(S=4096, add_sbias=False):
    NCH = S // 128   # s-chunks
    NT = S // 512    # s-tiles
    DC = D // 128    # d-chunks

    nc = bacc.Bacc(None, target_bir_lowering=False, debug=False)

    x8_d = nc.dram_tensor("x8", [S, D], FP8, kind="ExternalInput")
    xT8_d = nc.dram_tensor("xT8", [128, NT * DC * 512], FP8,
                           kind="ExternalInput")
    xT_d = nc.dram_tensor("xT", [D, S], BF16, kind="ExternalInput")
    bigw8_d = nc.dram_tensor("bigw8", [128, DC * GWP], FP8,
                             kind="ExternalInput")
    wvT_d = nc.dram_tensor("wvT", [128, DC * D], BF16, kind="ExternalInput")
    f2T_d = nc.dram_tensor("f2T", [128, DC * D], BF16, kind="ExternalInput")
    cw1T_d = nc.dram_tensor("cw1T", [128, DC * D], BF16, kind="ExternalInput")
    bv_d = nc.dram_tensor("bv", [D], F32, kind="ExternalInput")
    combb_d = nc.dram_tensor("combb", [D], F32, kind="ExternalInput")
    wgbn_d = nc.dram_tensor("wgbn", [1], F32, kind="ExternalInput")
    sbias_d = nc.dram_tensor("sbias", [H * M], F32, kind="ExternalInput")
    yT_d = nc.dram_tensor("yT", [D, S], BF16, kind="ExternalOutput")

    x8_ap = x8_d.ap()
    yT_ap = yT_d.ap()
    xT8_r = xT8_d.ap().rearrange("p (t dc s) -> p t dc s", t=NT, dc=DC)
    xT_r = xT_d.ap().rearrange("(dc p) s -> p dc s", p=128)
    bigw8_r = bigw8_d.ap().rearrange("p (dc g) -> p dc g", dc=DC)
    wvT_r = wvT_d.ap().rearrange("p (dc g) -> p dc g", dc=DC)
    f2T_r = f2T_d.ap().rearrange("p (dc g) -> p dc g", dc=DC)
    cw1T_r = cw1T_d.ap().rearrange("p (dc g) -> p dc g", dc=DC)
    combb_r = combb_d.ap().rearrange("(dc p) -> p dc", p=128)

    def bcast(ap, n):
        return bass.AP(tensor=ap.tensor, offset=ap.offset, ap=[[0, n]] + list(ap.ap))

    with tile.TileContext(nc) as tc:
        with tc.tile_pool(name="singles", bufs=1) as singles:
            # phase-1-critical loads first, on the sync queue
            bigw8_sb = singles.tile([128, DC, GWP], FP8)
            nc.sync.dma_start(bigw8_sb, bigw8_r)
            xT8_sb = singles.tile([128, NT, DC, 512], FP8)
            for t0 in range(3):
                nc.sync.dma_start(xT8_sb[:, t0], xT8_r[:, t0])
            # small constants on gpsimd queue
            combb_sb = singles.tile([128, DC], F32)
            nc.gpsimd.dma_start(combb_sb, combb_r)
            bvb_sb = singles.tile([64, D], F32)
            nc.gpsimd.dma_start(bvb_sb, bcast(bv_d.ap(), 64))
            wgbn_sb = singles.tile([128, 1], F32)
            nc.gpsimd.dma_start(wgbn_sb, bcast(wgbn_d.ap(), 128))
            sbias_sb = singles.tile([128, H * M], F32)
            nc.gpsimd.dma_start(sbias_sb, bcast(sbias_d.ap(), 128))
            ident = singles.tile([128, 128], BF16)
            make_identity(nc, ident)
            onesp = singles.tile([128, 2, 16], FP8)
            nc.vector.memset(onesp, 1.0)
            pT_all = singles.tile([128, 2, S], FP8)
            # deferred big loads (issued inside phase 1, gpsimd queue)
            xT_sb = singles.tile([128, DC, S], BF16)
            cw1T_sb = singles.tile([128, DC, D], BF16)
            wvT_sb = singles.tile([128, DC, D], BF16)
            f2T_sb = singles.tile([128, DC, D], BF16)

            # ---------------- phase 1: write-attention ----------------
            with (
                tc.tile_pool(name="ps1", bufs=1, space="PSUM") as ps1,
                tc.tile_pool(name="xin", bufs=3) as xin,
                tc.tile_pool(name="wk1", bufs=4) as wk1,
            ):
                ps_w = ps1.tile([64, 1536], F32, tag="w")

                def issue_pbig(c):
                    t, cc = c // 4, c % 4
                    pbig = ps1.tile([128, GW], F32, tag="big", bufs=3)
                    for j in range(DC // 2):
                        nc.tensor.matmul(
                            pbig,
                            lhsT=xT8_sb[:, t, 2 * j:2 * j + 2,
                                        cc * 128:(cc + 1) * 128],
                            rhs=bigw8_sb[:, 2 * j:2 * j + 2, 0:GW],
                            perf_mode=DR,
                            start=(j == 0), stop=(j == DC // 2 - 1),
                        )
                    return pbig

                def exps(c, pbig, pt):
                    es2, eg2, eh2 = pt
                    j = c % 2
                    if add_sbias:
                        nc.vector.tensor_add(
                            pbig[:, M + 1:GW], pbig[:, M + 1:GW], sbias_sb
                        )
                    nc.scalar.activation(es2[:, j, :], pbig[:, 0:M], AF.Exp)
                    nc.scalar.activation(eg2[:, j:j + 1], pbig[:, M:M + 1],
                                         AF.Exp, scale=-1.0, bias=wgbn_sb)
                    nc.scalar.activation(eh2[:, j], pbig[:, M + 1:GW], AF.Exp)

                def dve_pair(cp, pt, gcp):
                    es2, eg2, eh2 = pt
                    esum2 = wk1.tile([128, 2], F32, tag="esum")
                    nc.vector.reduce_sum(esum2, es2, axis=AX)
                    hs2 = wk1.tile([128, 2, H], F32, tag="hs")
                    nc.vector.reduce_sum(hs2, eh2, axis=AX)
                    den2 = wk1.tile([128, 2], F32, tag="den")
                    nc.vector.scalar_tensor_tensor(
                        out=den2, in0=eg2, scalar=1.0, in1=esum2,
                        op0=ALU.add, op1=ALU.mult)
                    r2 = wk1.tile([128, 2], F32, tag="r2")
                    nc.vector.reciprocal(r2, den2)
                    rh2 = wk1.tile([128, 2, H], F32, tag="rh")
                    nc.vector.reciprocal(rh2, hs2)
                    nc.gpsimd.tensor_mul(gcp, es2,
                                         r2.broadcast_to((128, 2, M)))
                    pn2 = wk1.tile([128, 2, H, M], BF16, tag="pn")
                    nc.vector.tensor_mul(pn2, eh2,
                                         rh2.broadcast_to((128, 2, H, M)))
                    return pn2

                def trans_pair(cp, pn2):
                    pnf = pn2.rearrange("p j h m -> p (j h m)")
                    ptr = ps1.tile([128, 512], BF16, tag="tr", bufs=2)
                    for jq in range(4):
                        nc.tensor.transpose(
                            ptr[:, jq * 128:(jq + 1) * 128],
                            pnf[:, jq * 128:(jq + 1) * 128], ident)
                    nc.vector.tensor_copy(
                        pT_all[:, :, cp * 256:(cp + 1) * 256].rearrange(
                            "p q (j u) -> p q j u", j=2),
                        ptr.rearrange("p (j q u) -> p q j u", j=2, q=2))

                def pe_tail(cp, gcp, xp):
                    st, sp = (cp == 0), (cp == NCH // 2 - 1)
                    nc.tensor.matmul(ps_w[:, 0:512], lhsT=gcp,
                                     rhs=xp[:, :, 0:512], perf_mode=DR,
                                     start=st, stop=sp)
                    nc.tensor.matmul(ps_w[:, 512:1024], lhsT=gcp,
                                     rhs=xp[:, :, 512:1024], perf_mode=DR,
                                     start=st, stop=sp)
                    nc.tensor.matmul(ps_w[:, 1024:1025], lhsT=gcp,
                                     rhs=onesp[:, :, 0:1], perf_mode=DR,
                                     start=st, stop=sp)

                # deferred-load schedule: (chunk -> dma issue) on the ACT
                # queue's DMA ring so the sync ring (xp/xT8) is undisturbed;
                # xT tiles 4-7 and cw1T stream during boundary/early phase 2
                def deferred_loads(c):
                    if c in (3, 5, 7, 9, 11, 13, 15, 17):
                        t = (c - 3) // 2
                        nc.sync.dma_start(
                            xT_sb[:, :, t * 256:(t + 1) * 256],
                            xT_r[:, :, t * 256:(t + 1) * 256])
                    elif c in (19, 21):
                        h = (c - 19) // 2
                        nc.sync.dma_start(
                            wvT_sb[:, 4 * h:4 * h + 4, :],
                            wvT_r[:, 4 * h:4 * h + 4, :])
                    elif c in (23, 25):
                        h = (c - 23) // 2
                        nc.sync.dma_start(
                            f2T_sb[:, 4 * h:4 * h + 4, :],
                            f2T_r[:, 4 * h:4 * h + 4, :])
                    elif c in (27, 29):
                        h = (c - 27) // 2
                        nc.sync.dma_start(
                            cw1T_sb[:, 4 * h:4 * h + 4, :],
                            cw1T_r[:, 4 * h:4 * h + 4, :])

                pairs = {}
                ptiles = {}
                chunks = {}
                pns = {}
                for c in range(NCH):
                    if c % 4 == 0 and c // 4 + 3 < NT:
                        t = c // 4 + 3
                        nc.sync.dma_start(xT8_sb[:, t], xT8_r[:, t])
                    if c % 2 == 0:
                        cp = c // 2
                        xp = xin.tile([128, 2, D], FP8, tag="xp")
                        nc.sync.dma_start(
                            xp, x8_ap[cp * 256:(cp + 1) * 256, :].rearrange(
                                "(j p) d -> p j d", p=128))
                        gcp = wk1.tile([128, 2, M], FP8, tag="gcp", bufs=3)
                        pairs[cp] = (gcp, xp)
                        es2 = wk1.tile([128, 2, M], BF16, tag="es", bufs=3)
                        eg2 = wk1.tile([128, 2], F32, tag="eg", bufs=3)
                        eh2 = wk1.tile([128, 2, H, M], BF16, tag="eh", bufs=3)
                        ptiles[cp] = (es2, eg2, eh2)
                    pbig = issue_pbig(c)
                    chunks[c] = pbig
                    if c >= 1:
                        exps(c - 1, chunks[c - 1], ptiles[(c - 1) // 2])
                        del chunks[c - 1]
                    if c >= 2 and c % 2 == 0:
                        cp = (c - 2) // 2
                        pns[cp] = dve_pair(cp, ptiles[cp], pairs[cp][0])
                        del ptiles[cp]
                    if c >= 3 and c % 2 == 1:
                        cp = (c - 3) // 2
                        trans_pair(cp, pns[cp])
                        del pns[cp]
                    deferred_loads(c)
                    if c >= 4 and c % 2 == 0:
                        cp = (c - 4) // 2
                        pe_tail(cp, *pairs[cp])
                        del pairs[cp]
                LP = NCH // 2 - 1  # last pair
                exps(NCH - 1, chunks[NCH - 1], ptiles[LP])
                pns[LP] = dve_pair(LP, ptiles[LP], pairs[LP][0])
                trans_pair(LP, pns[LP])
                pe_tail(LP - 1, *pairs[LP - 1])
                pe_tail(LP, *pairs[LP])

                # --- slot gate ---
                ssum = singles.tile([64, 1], F32)
                nc.vector.tensor_copy(ssum, ps_w[:, 1024:1025])
                sg = singles.tile([64, 1], F32)
                nc.vector.tensor_scalar_min(sg, ssum, 1.0)
                mv_bf = singles.tile([64, D], BF16)
                nc.vector.tensor_scalar_mul(mv_bf, ps_w[:, 0:D], sg)

            # ---------------- phase boundary: v and VF ----------------
            mvT_sb = singles.tile([128, DC, 64], BF16)
            vT_sb = singles.tile([128, DC, 64], BF16)
            v_sb = singles.tile([64, D], BF16)
            vf8 = singles.tile([128, 2, D], FP8)
            with tc.tile_pool(name="psB", bufs=1, space="PSUM") as psB:
                trB0 = psB.tile([128, DC, 64], BF16, tag="trb", bufs=2)
                for dc in range(DC):
                    nc.tensor.transpose(
                        trB0[:, dc, :], mv_bf[:, dc * 128:(dc + 1) * 128],
                        ident[0:64, 0:64])
                nc.vector.tensor_copy(mvT_sb, trB0)
                pv = psB.tile([64, D], F32, tag="v")
                for g2 in range(2):
                    for dc in range(DC):
                        nc.tensor.matmul(
                            pv[:, g2 * 512:(g2 + 1) * 512],
                            lhsT=mvT_sb[:, dc, :],
                            rhs=wvT_sb[:, dc, g2 * 512:(g2 + 1) * 512],
                            start=(dc == 0), stop=(dc == DC - 1),
                        )
                nc.vector.tensor_add(v_sb, pv, bvb_sb)
                trB1 = psB.tile([128, DC, 64], BF16, tag="trb", bufs=2)
                for dc in range(DC):
                    nc.tensor.transpose(
                        trB1[:, dc, :], v_sb[:, dc * 128:(dc + 1) * 128],
                        ident[0:64, 0:64])
                nc.vector.tensor_copy(vT_sb, trB1)
                for q in range(2):
                    pvf = psB.tile([128, D], F32, tag="vf", bufs=2)
                    for hh in range(2):
                        h = 2 * q + hh
                        for cc in range(2):
                            for g2 in range(2):
                                nc.tensor.matmul(
                                    pvf[hh * 64:(hh + 1) * 64,
                                        g2 * 512:(g2 + 1) * 512],
                                    lhsT=vT_sb[:, h * 2 + cc, :],
                                    rhs=f2T_sb[:, h * 2 + cc,
                                               g2 * 512:(g2 + 1) * 512],
                                    start=(cc == 0), stop=(cc == 1),
                                )
                    nc.vector.tensor_copy(vf8[:, q, :], pvf)

            # ---------------- phase 2: transposed output ----------------
            with (
                tc.tile_pool(name="ps2", bufs=2, space="PSUM") as ps2,
                tc.tile_pool(name="wk2", bufs=4) as wk2,
            ):
                for sh in range(2):
                    for dc in range(DC):
                        if sh == 0 and dc in (0, 2, 4, 6):
                            t = 4 + dc // 2
                            nc.gpsimd.dma_start(
                                xT_sb[:, :, t * 512:(t + 1) * 512],
                                xT_r[:, :, t * 512:(t + 1) * 512])
                        po = [ps2.tile([128, 1024], F32, tag=f"o{i}",
                                       name=f"po{i}") for i in range(2)]
                        for ci in range(DC + 1):
                            for st in range(4):
                                s0 = sh * 2048 + st * 512
                                out = po[st // 2][:, (st % 2) * 512:
                                                  (st % 2) * 512 + 512]
                                if ci < DC:
                                    nc.tensor.matmul(
                                        out,
                                        lhsT=cw1T_sb[:, ci,
                                                     dc * 128:(dc + 1) * 128],
                                        rhs=xT_sb[:, ci, s0:s0 + 512],
                                        start=(ci == 0), stop=False,
                                    )
                                else:
                                    nc.tensor.matmul(
                                        out,
                                        lhsT=vf8[:, :, dc * 128:(dc + 1) * 128],
                                        rhs=pT_all[:, :, s0:s0 + 512],
                                        perf_mode=DR,
                                        start=False, stop=True,
                                    )
                        ytile = wk2.tile([128, 4, 512], BF16, tag="yt")
                        for i in range(2):
                            nc.vector.tensor_scalar_add(
                                ytile[:, 2 * i:2 * i + 2, :], po[i],
                                combb_sb[:, dc:dc + 1])
                        nc.sync.dma_start(
                            yT_ap[dc * 128:(dc + 1) * 128,
                                  sh * 2048:(sh + 1) * 2048],
                            ytile,
                        )

    nc.compile()
    return nc


def prep_inputs(inputs, S=4096):
    """Host-side fusion + per-core shard maps."""
    f64 = np.float64
    bf = ml_dtypes.bfloat16
    f8 = ml_dtypes.float8_e4m3
    NT, DC = S // 512, D // 128
    x = np.asarray(inputs["x"], np.float32)
    mk = np.asarray(inputs["memory_keys"], np.float32)
    wg_w = np.asarray(inputs["wg_w"], np.float32)
    wg_b = np.asarray(inputs["wg_b"], np.float32)
    ipw = np.asarray(inputs["in_proj_w"], np.float32)
    ipb = np.asarray(inputs["in_proj_b"], np.float32)
    out_w = np.asarray(inputs["out_w"], np.float32)
    out_b = np.asarray(inputs["out_b"], np.float32)
    comb_w = np.asarray(inputs["comb_w"], np.float32)
    comb_b = np.asarray(inputs["comb_b"], np.float32)

    wq, wk, wv = ipw[:D], ipw[D:2 * D], ipw[2 * D:]
    bq, bk, bv = ipb[:D], ipb[D:2 * D], ipb[2 * D:]

    k_full = mk.astype(f64) @ wk.astype(f64).T + bk.astype(f64)      # (M, D)
    kh = k_full.reshape(M, H, DH)
    wqh = wq.astype(f64).reshape(H, DH, D)
    scl = 1.0 / np.sqrt(DH)
    FK = (np.einsum("mhd,hde->hme", kh, wqh) * scl).reshape(H * M, D)
    sbias = (np.einsum("hd,mhd->hm", bq.astype(f64).reshape(H, DH), kh)
             * scl).reshape(H * M)
    BIG_W = np.concatenate([mk.astype(f64), wg_w.astype(f64), FK], axis=0)

    fused2 = comb_w[:, D:].astype(f64) @ out_w.astype(f64)           # (D, D)
    combb = comb_b.astype(f64) + comb_w[:, D:].astype(f64) @ out_b.astype(f64)

    def preshuffle(wT, pad_to=None):
        # (D, G) -> (128, nc_*G): [p, dc*G+g] = wT[dc*128+p, g]
        nc_ = wT.shape[0] // 128
        G = wT.shape[1]
        if pad_to is not None and pad_to > G:
            wT = np.concatenate(
                [wT, np.zeros((wT.shape[0], pad_to - G), wT.dtype)], axis=1)
            G = pad_to
        return np.ascontiguousarray(
            wT.reshape(nc_, 128, G).transpose(1, 0, 2).reshape(128, nc_ * G))

    shared = {
        "bigw8": preshuffle(BIG_W.T.astype(np.float32),
                            pad_to=GWP).astype(f8),
        "wvT": preshuffle(np.ascontiguousarray(wv.T)).astype(bf),
        "f2T": preshuffle(np.ascontiguousarray(fused2.T)).astype(bf),
        "cw1T": preshuffle(np.ascontiguousarray(comb_w[:, :D].T)).astype(bf),
        "bv": bv.astype(np.float32),
        "combb": combb.astype(np.float32),
        "wgbn": (-wg_b).astype(np.float32),
        "sbias": sbias.astype(np.float32),
    }
    add_sbias = bool(np.any(shared["sbias"] != 0))

    in_maps = []
    for b in range(B):
        xb = x[b, :S]
        xbT = np.ascontiguousarray(xb.T)                  # (D, S)
        # xT8 layout [p, t, dc, s']: value = xT[dc*128+p, t*512+s']
        xT8 = (xbT.reshape(DC, 128, NT, 512)
               .transpose(1, 2, 0, 3).reshape(128, NT * DC * 512))
        m = dict(shared)
        m["x8"] = xb.astype(f8)
        m["xT8"] = np.ascontiguousarray(xT8).astype(f8)
        m["xT"] = xbT.astype(bf)
        in_maps.append(m)
    return in_maps, add_sbias


def kernel(_trace=False, _S=4096, **inputs):
    in_maps, add_sbias = prep_inputs(inputs, S=_S)
    nc = build_program(S=_S, add_sbias=add_sbias)
    kw = {}
    if _trace:
        kw = dict(trace=True, trace_cores=list(range(N_CORES)))
    res = run_bass_kernel_spmd(nc, in_maps, list(range(N_CORES)), **kw)
    y = np.stack(
        [np.asarray(res.results[i]["yT"]).astype(np.float32).T
         for i in range(N_CORES)],
        axis=0,
    )
    if _trace:
        return y, res
    return y
